# revision 1
# baseline (speedup 1.0000x reference)
"""Causal self-attention (B=2, T=2048, C=1024, H=16) on 8 Trainium2 cores.

Sharding: tensor-parallel over heads (2 heads/core). Each core computes
QKV projection for its heads, causal attention, and a partial c_proj
output; partials are summed on the host (b_proj is added by core 0 only).

Per-core dataflow (everything kept "K-major" so no activation transposes
are needed on the critical path):
  xT [C, B*T]  (host pre-transposes x)
  qT/kT/vT [128, B*T] = W_local^T @ x + b      (PE, fp32r)
  S^T tile [k 128, q 512] = K @ Q^T            (PE)  -- causal tiles only
  E^T = exp(S^T/8) * causal_mask               (ACT + DVE)
  y'^T [65, q 512] += [v | 1]^T @ E^T          (PE; row 64 = softmax sums)
  y_norm^T = y'^T[0:64] * bcast(1/sums)        (PE rank-1 bcast + DVE)
  partial^T [c 128, row 512] = Wp_local^T-ish  (PE) + b_proj  -> DRAM

fp32r is used for all matmuls (full PE rate at free-dim >= 256, ~1e-4
rel err vs fp32). The BIR verifier requires fp32r operands to come from
fp32r-typed producers: DRAM inputs are declared fp32r (same bits as
fp32) so plain HWDGE DMAs satisfy it; intermediates are written as
fp32r by ACT/DVE ops.
"""

import numpy as np

import concourse.bass as bass
import concourse.tile as tile
from concourse import bacc, mybir
from concourse.bass_utils import run_bass_kernel_spmd
from concourse.masks import make_identity

F32 = mybir.dt.float32
F32R = mybir.dt.float32r

B, T, C, H = 2, 2048, 1024, 16
HS = C // H            # 64 head dim
NCORES = 8
HL = H // NCORES       # 2 local heads
LC = HL * HS           # 128 local q/k/v cols
R = B * T              # 4096 rows (b, t)
KC = C // 128          # 8 contraction chunks for projections
QT = 512               # attention q tile (free dim)
NQT = T // QT          # 4
KA = 128               # attention k chunk (partition dim)
NKA = T // KA          # 16
RT = 512               # row tile for projections
NRT = R // RT          # 8
NCC = C // 128         # 8 c_proj output chunks


def build_program():
    nc = bacc.Bacc("TRN2", target_bir_lowering=False, debug=False,
                   num_devices=NCORES)

    xT = nc.dram_tensor("xT", [C, R], F32R, kind="ExternalInput").ap()
    wqkv = nc.dram_tensor("wqkv", [C, 3 * LC], F32R, kind="ExternalInput").ap()
    bqkv = nc.dram_tensor("bqkv", [3 * LC], F32, kind="ExternalInput").ap()
    wp = nc.dram_tensor("wp", [LC, C], F32R, kind="ExternalInput").ap()
    bp = nc.dram_tensor("bp", [C], F32, kind="ExternalInput").ap()
    trimask = nc.dram_tensor("trimask", [KA, KA], F32R, kind="ExternalInput").ap()
    outT = nc.dram_tensor("outT", [C, R], F32, kind="ExternalOutput").ap()

    with tile.TileContext(nc) as tc:
        with (
            tc.tile_pool(name="consts", bufs=1) as consts,
            tc.tile_pool(name="weights", bufs=1) as weights,
            tc.tile_pool(name="qkvT", bufs=1) as qkvT_pool,
            tc.tile_pool(name="xs", bufs=3) as xs_pool,
            tc.tile_pool(name="vp", bufs=2 * NKA) as vp_pool,
            tc.tile_pool(name="et", bufs=10) as et_pool,
            tc.tile_pool(name="ysb", bufs=3) as ysb_pool,
            tc.tile_pool(name="rec", bufs=2) as rec_pool,
            tc.tile_pool(name="osb", bufs=8) as osb_pool,
            tc.tile_pool(name="dscr", bufs=4, space="DRAM") as dscr_pool,
            tc.tile_pool(name="mm512", bufs=3, space="PSUM") as mm512_pool,
            tc.tile_pool(name="ytps", bufs=2, space="PSUM") as ytps_pool,
            tc.tile_pool(name="smps", bufs=3, space="PSUM") as smps_pool,
        ):
            # ---- constants ----
            identity = consts.tile([128, 128], F32)
            make_identity(nc, identity)
            ones64_f = consts.tile([1, HS], F32)
            nc.vector.memset(ones64_f, 1.0)
            ones64 = consts.tile([1, HS], F32R)
            nc.vector.tensor_copy(ones64, ones64_f)
            ones_col = consts.tile([128, 1], F32)
            nc.vector.memset(ones_col, 1.0)
            tri_sb = consts.tile([KA, KA], F32R)
            bqkv_sb = consts.tile([128, 3], F32)
            bp_sb = consts.tile([128, NCC], F32)

            # ---- weights (fp32r-typed DRAM, plain HWDGE loads) ----
            wq_sb = weights.tile([128, KC, 3 * LC], F32R)
            wq_r = wqkv.rearrange("(kc p) n -> p kc n", p=128)
            nc.sync.dma_start(out=wq_sb[:, 0:2], in_=wq_r[:, 0:2])
            nc.sync.dma_start(
                out=bqkv_sb, in_=bqkv.rearrange("(j p) -> p j", p=128))
            wp_sb = weights.tile([LC, C], F32R)

            def load_consts():
                nc.sync.dma_start(out=tri_sb, in_=trimask)
                nc.sync.dma_start(
                    out=bp_sb, in_=bp.rearrange("(j p) -> p j", p=128))
                nc.sync.dma_start(out=wp_sb, in_=wp)

            # ---- phase 1: QKV projection (transposed outputs) ----
            qT_s = qkvT_pool.tile([LC, R], F32R, tag="qT")
            kT_s = qkvT_pool.tile([LC, R], F32R, tag="kT")
            vT_s = qkvT_pool.tile([LC, R], F32R, tag="vT")
            dst_tiles = [qT_s, kT_s, vT_s]

            def qkv_load(rt):
                x_sb = xs_pool.tile([128, KC, RT], F32R, tag="xs", name=f"x_sb_rt{rt}")
                x_r = xT[:, rt * RT:(rt + 1) * RT].rearrange(
                    "(kc p) r -> p kc r", p=128)
                if rt == 0:
                    for kc in range(0, KC, 2):
                        nc.scalar.dma_start(out=x_sb[:, kc:kc + 2],
                                            in_=x_r[:, kc:kc + 2])
                else:
                    nc.scalar.dma_start(out=x_sb[:, 0:KC // 2],
                                        in_=x_r[:, 0:KC // 2])
                    nc.scalar.dma_start(out=x_sb[:, KC // 2:],
                                        in_=x_r[:, KC // 2:])
                return x_sb

            def qkv_compute(rt, x_sb):
                if rt == 0:
                    # kc-outer for the very first tile: matmuls start as soon
                    # as the first x/w chunk lands instead of after all 8
                    pss = [mm512_pool.tile([128, RT], F32, tag="mm512",
                                           name=f"qkv_ps_rt0c{col}")
                           for col in range(3)]
                    for kc in range(KC):
                        for col in range(3):
                            nc.tensor.matmul(
                                pss[col],
                                wq_sb[:, kc, col * LC:(col + 1) * LC],
                                x_sb[:, kc, :],
                                start=(kc == 0),
                                stop=(kc == KC - 1),
                            )
                    for col in range(3):
                        nc.vector.tensor_scalar_add(
                            dst_tiles[col][:, 0:RT],
                            pss[col],
                            bqkv_sb[:, col:col + 1],
                        )
                    return
                for col in range(3):
                    ps = mm512_pool.tile([128, RT], F32, tag="mm512",
                                         name=f"qkv_ps_rt{rt}c{col}")
                    for kc in range(KC):
                        nc.tensor.matmul(
                            ps,
                            wq_sb[:, kc, col * LC:(col + 1) * LC],
                            x_sb[:, kc, :],
                            start=(kc == 0),
                            stop=(kc == KC - 1),
                        )
                    # PSUM -> SBUF with per-partition bias add, rounding to f32r
                    nc.vector.tensor_scalar_add(
                        dst_tiles[col][:, rt * RT:(rt + 1) * RT],
                        ps,
                        bqkv_sb[:, col:col + 1],
                    )

            def proj_rowtile(rt, tail):
                """c_proj partial for row tile rt (needs ynT rows complete)."""
                half = RT // 2
                for cc in range(NCC):
                    if tail and cc % 2 == 1:
                        pps = smps_pool.tile([128, RT], F32, tag="sm",
                                             name=f"pps_rt{rt}c{cc}")
                    else:
                        pps = mm512_pool.tile([128, RT], F32, tag="mm512",
                                              name=f"pps_rt{rt}c{cc}")
                    nc.tensor.matmul(
                        pps,
                        wp_sb[:, cc * 128:(cc + 1) * 128],
                        ynT_s[:, rt * RT:(rt + 1) * RT],
                        start=True,
                        stop=True,
                    )
                    o_sb = osb_pool.tile([128, RT], F32, tag="osb",
                                         name=f"o_sb_rt{rt}c{cc}")
                    if tail:
                        # both engines are idle in the tail: split for slot
                        # turnover
                        nc.scalar.activation(
                            o_sb[:, 0:half], pps[:, 0:half],
                            mybir.ActivationFunctionType.Identity,
                            bias=bp_sb[:, cc:cc + 1],
                        )
                        nc.vector.tensor_scalar_add(
                            o_sb[:, half:RT], pps[:, half:RT],
                            bp_sb[:, cc:cc + 1])
                    else:
                        # overlapped with exp-bound attention: keep ACT free
                        nc.vector.tensor_scalar_add(o_sb, pps,
                                                    bp_sb[:, cc:cc + 1])
                    nc.sync.dma_start(
                        out=outT[cc * 128:(cc + 1) * 128,
                                 rt * RT:(rt + 1) * RT],
                        in_=o_sb,
                    )

            # ---- phase 2: attention per (b, h), interleaved with QKV/proj ----
            ynT_s = qkvT_pool.tile([LC, R], F32R, tag="ynT")
            x0 = qkv_load(0)
            nc.sync.dma_start(out=wq_sb[:, 2:4], in_=wq_r[:, 2:4])
            nc.sync.dma_start(out=wq_sb[:, 4:KC], in_=wq_r[:, 4:KC])
            qkv_compute(0, x0)
            x1 = qkv_load(1)
            load_consts()
            qkv_compute(1, x1)
            for rt in range(2, NRT // 2):
                qkv_compute(rt, qkv_load(rt))
            for b in range(B):
                base = b * T
                if b + 1 < B:
                    for rt in range((b + 1) * NRT // 2, (b + 2) * NRT // 2):
                        qkv_compute(rt, qkv_load(rt))
                for h in range(HL):
                    hsl = slice(h * HS, (h + 1) * HS)
                    vps = []

                    # qt-outer: only one y' accumulator live at a time
                    for qt in range(NQT):
                        # v' tiles [k 128, 64 v-cols | ones] for the k chunks
                        # this qt introduces -- lazy prep keeps (h, qt)
                        # dependent only on QKV row tiles <= qt
                        for kc in range(qt * (QT // KA),
                                        (qt + 1) * (QT // KA)):
                            vp = vp_pool.tile([KA, HS + 1], F32R,
                                              name=f"vp_b{b}h{h}k{kc}",
                                              tag="vp")
                            tp = mm512_pool.tile([KA, HS], F32, tag="mm512",
                                                 name=f"tp_b{b}h{h}k{kc}")
                            nc.tensor.transpose(
                                tp,
                                vT_s[hsl,
                                     base + kc * KA: base + (kc + 1) * KA]
                                .bitcast(F32),
                                identity[hsl, hsl],
                            )
                            nc.vector.tensor_copy(vp[:, 0:HS], tp)
                            nc.gpsimd.tensor_copy(vp[:, HS:HS + 1], ones_col)
                            vps.append(vp)
                        yp = ytps_pool.tile([HS + 1, QT], F32, tag="yt",
                                            name=f"yt_b{b}h{h}q{qt}")
                        nka_q = (qt + 1) * (QT // KA)
                        for kc in range(nka_q):
                            diag = (kc * KA // QT == qt)
                            sps = smps_pool.tile(
                                [KA, QT], F32, tag="sm",
                                name=f"sps_b{b}h{h}q{qt}k{kc}")
                            nc.tensor.matmul(
                                sps,
                                kT_s[hsl,
                                     base + kc * KA: base + (kc + 1) * KA],
                                qT_s[hsl,
                                     base + qt * QT: base + (qt + 1) * QT],
                                start=True,
                                stop=True,
                            )
                            et = et_pool.tile([KA, QT], F32R, tag="et",
                                              name=f"et_b{b}h{h}q{qt}k{kc}")
                            # columns < off of a diagonal tile are fully
                            # masked; skip them entirely (the AV matmul
                            # accumulates only the [off, QT) span).
                            off = kc * KA - qt * QT if diag else 0
                            nc.scalar.activation(
                                et[:, off:QT], sps[:, off:QT],
                                mybir.ActivationFunctionType.Exp,
                                scale=1.0 / np.sqrt(HS).item(),
                            )
                            if diag:
                                # [off, off+128) is the triangular block
                                nc.gpsimd.tensor_mul(
                                    et[:, off:off + KA],
                                    et[:, off:off + KA],
                                    tri_sb,
                                )
                            nc.tensor.matmul(
                                yp[:, off:QT],
                                vps[kc],
                                et[:, off:QT],
                                start=(kc == 0),
                                stop=(kc == nka_q - 1),
                            )

                        # normalize: y_norm^T = y'^T[0:64] * bcast(1 / sums)
                        yts = ysb_pool.tile([HS + 1, QT], F32, tag="yts",
                                            name=f"yts_b{b}h{h}q{qt}")
                        nc.vector.tensor_copy(yts, yp)
                        rec = rec_pool.tile([1, QT], F32R, tag="rec",
                                            name=f"rec_b{b}h{h}q{qt}")
                        with nc.allow_low_precision(
                                reason="fp32r reciprocal: ~1e-4 rel err ok"):
                            nc.vector.reciprocal(rec, yts[HS:HS + 1, :])
                        if qt == NQT - 1 and h == HL - 1:
                            # end of batch: nothing else keeps PE busy, and
                            # the DRAM-bounce latency would gate the final
                            # c_proj row tile -- use a rank-1 PE broadcast
                            bcp = smps_pool.tile([HS, QT], F32, tag="sm",
                                                 name=f"bcp_b{b}h{h}q{qt}")
                            nc.tensor.matmul(bcp, ones64, rec,
                                             start=True, stop=True)
                            nc.vector.tensor_mul(
                                ynT_s[hsl,
                                      base + qt * QT: base + (qt + 1) * QT],
                                yts[0:HS, :],
                                bcp,
                            )
                        else:
                            bcs = ysb_pool.tile([HS, QT], F32R, tag="bcs",
                                                name=f"bcs_b{b}h{h}q{qt}")
                            recd = dscr_pool.tile([1, QT], F32R, tag="recd",
                                                  name=f"recd_b{b}h{h}q{qt}")
                            nc.sync.dma_start(out=recd, in_=rec)
                            rec_bcast = bass.AP(
                                tensor=recd.tensor, offset=recd.offset,
                                ap=[[0, HS]] + [list(d) for d in recd.ap[1:]])
                            nc.sync.dma_start(out=bcs, in_=rec_bcast)
                            nc.vector.tensor_mul(
                                ynT_s[hsl,
                                      base + qt * QT: base + (qt + 1) * QT],
                                yts[0:HS, :],
                                bcs,
                            )
                        # c_proj row tiles interleave into the last head's
                        # attention, one qt behind the normalize that feeds
                        # them, so the PE never waits on the bcast chain and
                        # output DMA spreads across the attention window.
                        if h == HL - 1 and qt > 0:
                            proj_rowtile(b * NRT // 2 + qt - 1,
                                         tail=(b == B - 1 and qt == NQT - 1))

                # last row tile of this batch after its attention finishes
                proj_rowtile(b * NRT // 2 + NQT - 1, tail=(b == B - 1))

    nc.compile()
    return nc


_NC = None


def _get_nc():
    global _NC
    if _NC is None:
        _NC = build_program()
    return _NC


def make_in_maps(x, W_attn, b_attn, W_proj, b_proj):
    x = np.asarray(x, np.float32)
    W_attn = np.asarray(W_attn, np.float32)
    b_attn = np.asarray(b_attn, np.float32)
    W_proj = np.asarray(W_proj, np.float32)
    b_proj = np.asarray(b_proj, np.float32)

    xT = np.ascontiguousarray(x.reshape(R, C).T)
    tri = np.triu(np.ones((KA, KA), np.float32))  # [kk, j]: 1 if j >= kk
    zeros_bp = np.zeros_like(b_proj)

    in_maps = []
    for core in range(NCORES):
        g0 = core * HL * HS  # first local column in head space
        cols = slice(g0, g0 + LC)
        w_local = np.concatenate(
            [W_attn[:, i * C:(i + 1) * C][:, cols] for i in range(3)], axis=1)
        b_local = np.concatenate(
            [b_attn[i * C:(i + 1) * C][cols] for i in range(3)])
        in_maps.append({
            "xT": xT,
            "wqkv": np.ascontiguousarray(w_local),
            "bqkv": np.ascontiguousarray(b_local),
            "wp": np.ascontiguousarray(W_proj[cols, :]),
            "bp": b_proj if core == 0 else zeros_bp,
            "trimask": tri,
        })
    return in_maps


def kernel(x, W_attn, b_attn, W_proj, b_proj):
    nc = _get_nc()
    in_maps = make_in_maps(x, W_attn, b_attn, W_proj, b_proj)
    res = run_bass_kernel_spmd(nc, in_maps, list(range(NCORES)))
    acc = res.results[0]["outT"].copy()
    for corer in res.results[1:]:
        acc += corer["outT"]
    return np.ascontiguousarray(acc.T).reshape(B, T, C)



# revision 6
# speedup vs baseline: 1.0200x; 1.0200x over previous
"""Causal self-attention (B=2, T=2048, C=1024, H=16) on 8 Trainium2 cores.

Sharding: tensor-parallel over heads (2 heads/core). Each core computes the
QKV projection for its heads, causal attention, and a partial c_proj output;
partials (and b_proj) are summed on the host.

v2 dataflow — natural-orientation AV so every matmul runs with a full
128-partition output and the cost-model-minimal free size:
  xT fp16 [C, B*T]                       (host pre-transposes x)
  qT/kT [128, B*T] fp16 = Wqk^T @ x + b  (PE, 512-free tiles)
  v_nat [k 128, 65] fp16 = x^T @ Wv | 1  (PE, 64-free tiles, bias via rank-1)
  S^T [k 128, q<=1024] f32 = K Q^T       (PE; causal tiles only, diag-trimmed)
  et = exp(S^T/8) fp16                   (ACT; tri-mask on diag via Pool)
  yp [q 128, 65] f32 += et_j^T @ v_nat   (PE; col 64 accumulates softmax sums)
  y = yp[:,0:64] * recip(yp[:,64]) f32r  (DVE per-partition scalars, no bcast)
  ynT [64, 128] = PE-transpose(y)        (f32r, 80ns each)
  out^T [c 128, rows] f32 = Wp^T @ ynT   (PE, f32r) -> fp16 staging -> DRAM

fp16 is used for x/W/q/k/et/v (errors land before softmax or are ~1e-3);
the AV accumulation, normalize, and c_proj run in f32/f32r.
"""

import numpy as np

import concourse.bass as bass
import concourse.tile as tile
from concourse import bacc, mybir
from concourse.bass_utils import run_bass_kernel_spmd
from concourse.masks import make_identity

F32 = mybir.dt.float32
F32R = mybir.dt.float32r
F16 = mybir.dt.float16

B, T, C, H = 2, 2048, 1024, 16
HS = C // H            # 64 head dim
NCORES = 8
HL = H // NCORES       # 2 local heads
LC = HL * HS           # 128 local q (or k, or v) channels
R = B * T              # 4096 rows
KC = C // 128          # 8 contraction chunks for projections
RT = 512               # row tile for qk-proj / c_proj
NRT = R // RT          # 8
QW = 1024              # attention q window (S/exp tile width)
NQW = T // QW          # 2 per batch
KA = 128               # attention k chunk (partition dim)
NJ = T // KA           # 16 q-subchunks (=k chunks) per batch
NCC = C // 128         # 8 c_proj output chunks


def build_program():
    nc = bacc.Bacc("TRN2", target_bir_lowering=False, debug=False,
                   num_devices=NCORES)

    xT = nc.dram_tensor("xT", [C, R], F16, kind="ExternalInput").ap()
    wqk = nc.dram_tensor("wqk", [C, 2 * LC], F16, kind="ExternalInput").ap()
    wv = nc.dram_tensor("wv", [C, HL, HS], F16, kind="ExternalInput").ap()
    wp = nc.dram_tensor("wp", [LC, C], F32R, kind="ExternalInput").ap()
    bqk = nc.dram_tensor("bqk", [2 * LC], F32, kind="ExternalInput").ap()
    bv = nc.dram_tensor("bv", [1, LC], F16, kind="ExternalInput").ap()
    trimask = nc.dram_tensor("trimask", [KA, KA], F16, kind="ExternalInput").ap()
    outT = nc.dram_tensor("outT", [C, R], F16, kind="ExternalOutput").ap()

    with tile.TileContext(nc) as tc:
        with (
            tc.tile_pool(name="consts", bufs=1) as consts,
            tc.tile_pool(name="weights", bufs=1) as weights,
            tc.tile_pool(name="big", bufs=1) as big,
            tc.tile_pool(name="xs", bufs=3) as xs_pool,
            tc.tile_pool(name="et", bufs=18) as et_pool,
            tc.tile_pool(name="ysb", bufs=4) as ysb_pool,
            tc.tile_pool(name="rec", bufs=4) as rec_pool,
            tc.tile_pool(name="osb", bufs=2) as osb_pool,
            tc.tile_pool(name="sps", bufs=2, space="PSUM") as sps_pool,
            tc.tile_pool(name="ypb", bufs=1, space="PSUM") as ypb_pool,
            tc.tile_pool(name="qkps", bufs=2, space="PSUM") as qkps_pool,
            tc.tile_pool(name="vps", bufs=1, space="PSUM") as vps_pool,
        ):
            lp = nc.allow_low_precision(
                reason="fp16/f32r attention pipeline; ~1e-3 rel err validated")
            lp.__enter__()

            # ---- constants ----
            identity = consts.tile([128, 128], F32R)
            make_identity(nc, identity)
            ones1 = consts.tile([1, 128], F16)
            nc.vector.memset(ones1, 1.0)
            tri_sb = consts.tile([KA, KA], F16)
            bqk_sb = consts.tile([128, 2], F32)
            bv_sb = consts.tile([1, LC], F16)

            # ---- weights ----
            wqk_sb = weights.tile([128, KC, 2 * LC], F16)
            wqk_r = wqk.rearrange("(kc p) n -> p kc n", p=128)
            wv_sb = weights.tile([128, KC, HL, HS], F16)
            wv_r = wv.rearrange("(kc p) h n -> p kc h n", p=128)
            wp_sb = weights.tile([LC, C], F32R)

            def load_consts():
                nc.sync.dma_start(out=tri_sb, in_=trimask)
                nc.sync.dma_start(
                    out=bqk_sb, in_=bqk.rearrange("(j p) -> p j", p=128))
                nc.sync.dma_start(out=bv_sb, in_=bv)
                nc.sync.dma_start(out=wv_sb, in_=wv_r)
                nc.sync.dma_start(out=wp_sb, in_=wp)

            # ---- persistent activations ----
            qT_s = big.tile([LC, R], F16, tag="qT")
            kT_s = big.tile([LC, R], F16, tag="kT")
            ynT_s = big.tile([LC, R], F32R, tag="ynT")
            # v_nat[:, b, h, kci, 0:64] = v rows; col 64 = 1.0 (softmax sums)
            v_nat = big.tile([KA, B, HL, NJ, HS + 1], F16, tag="vnat")

            def qkv_load(rt):
                x_sb = xs_pool.tile([128, KC, RT], F16, tag="xs",
                                    name=f"x_sb_rt{rt}")
                x_r = xT[:, rt * RT:(rt + 1) * RT].rearrange(
                    "(kc p) r -> p kc r", p=128)
                if rt == 0:
                    nc.sync.dma_start(out=x_sb[:, 0:1], in_=x_r[:, 0:1])
                    nc.sync.dma_start(out=x_sb[:, 1:KC], in_=x_r[:, 1:KC])
                else:
                    nc.sync.dma_start(out=x_sb, in_=x_r)
                return x_sb

            def qk_proj(rt, x_sb):
                """q/k projection for row tile rt -> qT_s/kT_s (fp16)."""
                span = slice(rt * RT, (rt + 1) * RT)
                if rt == 0:
                    # kc-outer so matmuls start as soon as the first x/w
                    # chunk lands
                    pss = [qkps_pool.tile([128, RT], F32, tag="qkps",
                                          name=f"qk_ps_rt0c{col}")
                           for col in range(2)]
                    for kc in range(KC):
                        for col in range(2):
                            nc.tensor.matmul(
                                pss[col],
                                wqk_sb[:, kc, col * LC:(col + 1) * LC],
                                x_sb[:, kc, :],
                                start=(kc == 0), stop=(kc == KC - 1),
                            )
                    for col, dst in ((0, qT_s), (1, kT_s)):
                        nc.vector.tensor_scalar_add(
                            dst[:, span], pss[col], bqk_sb[:, col:col + 1])
                    return
                for col, dst in ((0, qT_s), (1, kT_s)):
                    ps = qkps_pool.tile([128, RT], F32, tag="qkps",
                                        name=f"qk_ps_rt{rt}c{col}")
                    for kc in range(KC):
                        nc.tensor.matmul(
                            ps,
                            wqk_sb[:, kc, col * LC:(col + 1) * LC],
                            x_sb[:, kc, :],
                            start=(kc == 0), stop=(kc == KC - 1),
                        )
                    nc.vector.tensor_scalar_add(
                        dst[:, span], ps, bqk_sb[:, col:col + 1])

            _vslot = [0]

            def v_proj(rt, x_sb):
                """v projection in natural layout for row tile rt."""
                b = (rt * RT) // T
                for h in range(HL):
                    for ch in range(RT // KA):
                        kci = (rt * RT - b * T) // KA + ch
                        s = _vslot[0]
                        _vslot[0] ^= 1
                        vp = vps_pool.tile([128, 2 * HS], F32, tag="vps")
                        slot = vp[:, s * HS:(s + 1) * HS]
                        # rank-1 bias: ones(128) x bv_h, then accumulate x@Wv
                        nc.tensor.matmul(
                            slot, ones1, bv_sb[:, h * HS:(h + 1) * HS],
                            start=True, stop=False)
                        for kc in range(KC):
                            nc.tensor.matmul(
                                slot,
                                x_sb[:, kc, ch * KA:(ch + 1) * KA],
                                wv_sb[:, kc, h, :],
                                start=False, stop=(kc == KC - 1),
                            )
                        nc.vector.tensor_copy(
                            v_nat[:, b, h, kci, 0:HS], slot)

            def qkv_tile(rt):
                x_sb = qkv_load(rt)
                qk_proj(rt, x_sb)
                v_proj(rt, x_sb)

            _oslot = [0]

            def c_proj(rt, lo, hi, tail=False):
                """c_proj partial for rows [rt*RT+lo, rt*RT+hi) -> DRAM."""
                w = hi - lo
                o_sb = osb_pool.tile([128, NCC, RT], F16, tag="osb",
                                     name=f"o_sb_rt{rt}_{lo}")
                for cc in range(NCC):
                    pps = qkps_pool.tile([128, RT], F32, tag="qkps",
                                         name=f"pps_rt{rt}_{lo}c{cc}")
                    nc.tensor.matmul(
                        pps[:, 0:w],
                        wp_sb[:, cc * 128:(cc + 1) * 128],
                        ynT_s[:, rt * RT + lo: rt * RT + hi],
                        start=True, stop=True,
                    )
                    if tail:
                        # both engines idle in the tail: split for turnover
                        nc.vector.tensor_copy(
                            o_sb[:, cc, 0:w // 2], pps[:, 0:w // 2])
                        nc.gpsimd.tensor_copy(
                            o_sb[:, cc, w // 2:w], pps[:, w // 2:w])
                    elif cc % 2 == 0:
                        nc.vector.tensor_copy(o_sb[:, cc, 0:w], pps[:, 0:w])
                    else:
                        nc.gpsimd.tensor_copy(o_sb[:, cc, 0:w], pps[:, 0:w])
                nc.sync.dma_start(
                    out=outT.rearrange("(cc p) r -> p cc r", p=128)[
                        :, :, rt * RT + lo: rt * RT + hi],
                    in_=o_sb[:, :, 0:w],
                )

            # ---- attention ----
            _yslot = [0]
            _tslot = [0]

            def finalize_j(b, h, j, ypb, yslot):
                """normalize, transpose and store q-subchunk j of head h."""
                base = b * T
                yp = ypb[:, yslot * (HS + 1):(yslot + 1) * (HS + 1)]
                rec = rec_pool.tile([128, 1], F32, tag="rec",
                                    name=f"rec_b{b}h{h}j{j}")
                nc.vector.reciprocal(rec, yp[:, HS:HS + 1])
                y_sb = ysb_pool.tile([128, HS], F32R, tag="ysb",
                                     name=f"y_b{b}h{h}j{j}")
                nc.vector.tensor_scalar_mul(y_sb, yp[:, 0:HS], rec)
                ts = _tslot[0]
                _tslot[0] ^= 1
                tdst = ypb[0:HS, 2 * (HS + 1) + ts * 128:
                           2 * (HS + 1) + (ts + 1) * 128].bitcast(F32R)
                nc.tensor.transpose(tdst, y_sb, identity)
                hsl = slice(h * HS, (h + 1) * HS)
                if j % 2 == 0:
                    nc.vector.tensor_copy(
                        ynT_s[hsl, base + j * KA: base + (j + 1) * KA], tdst)
                else:
                    nc.gpsimd.tensor_copy(
                        ynT_s[hsl, base + j * KA: base + (j + 1) * KA], tdst)

            def attn_window(b, h, qw, hooks):
                """S/exp/AV for q window qw of head h, batch b.

                hooks: dict j_global -> list of thunks interleaved after that
                q-subchunk's AV burst is issued.
                """
                base = b * T
                hsl = slice(h * HS, (h + 1) * HS)
                q0 = qw * QW
                nkc = (qw + 1) * (QW // KA)
                ets = []
                ypb = ypb_pool.tile([128, 512], F32, tag="ypb",
                                    name=f"ypb_b{b}h{h}q{qw}")
                for kc in range(nkc):
                    diag = (kc * KA >= q0)
                    off = kc * KA - q0 if diag else 0
                    sps = sps_pool.tile([KA, QW], F32, tag="sps",
                                        name=f"sps_b{b}h{h}q{qw}k{kc}")
                    nc.tensor.matmul(
                        sps[:, off:QW],
                        kT_s[hsl, base + kc * KA: base + (kc + 1) * KA],
                        qT_s[hsl, base + q0 + off: base + q0 + QW],
                        start=True, stop=True,
                    )
                    et = et_pool.tile([KA, QW], F16, tag="et",
                                      name=f"et_b{b}h{h}q{qw}k{kc}")
                    nc.scalar.activation(
                        et[:, off:QW], sps[:, off:QW],
                        mybir.ActivationFunctionType.Exp,
                        scale=1.0 / np.sqrt(HS).item(),
                    )
                    if diag:
                        nc.gpsimd.tensor_mul(
                            et[:, off:off + KA], et[:, off:off + KA], tri_sb)
                    ets.append(et)
                    if diag:
                        # q-subchunk j == kc is now complete: AV burst
                        j = kc
                        ys = _yslot[0]
                        _yslot[0] ^= 1
                        yp = ypb[:, ys * (HS + 1):(ys + 1) * (HS + 1)]
                        jw = j * KA - q0
                        for k2 in range(j + 1):
                            nc.tensor.matmul(
                                yp,
                                ets[k2][:, jw:jw + KA],
                                v_nat[:, b, h, k2, :],
                                start=(k2 == 0), stop=(k2 == j),
                            )
                        finalize_j(b, h, j, ypb, ys)
                    for hook in hooks.get((qw, kc), ()):
                        hook()

            # ---- schedule ----
            x0 = qkv_load(0)
            nc.sync.dma_start(out=wqk_sb[:, 0:2], in_=wqk_r[:, 0:2])
            nc.sync.dma_start(out=wqk_sb[:, 2:KC], in_=wqk_r[:, 2:KC])
            load_consts()
            nc.vector.memset(v_nat[:, :, :, :, HS:HS + 1], 1.0)
            qk_proj(0, x0)
            v_proj(0, x0)
            for rt in range(1, 4):
                qkv_tile(rt)

            # b0 h0: interleave b1 qkv tiles
            attn_window(0, 0, 0, {(0, 3): [lambda: qkv_tile(4)]})
            attn_window(0, 0, 1, {(1, 5): [lambda: qkv_tile(5)]})
            # b0 h1: c_proj for b0 rows as they complete + last b1 qkv
            attn_window(0, 1, 0, {
                (0, 2): [lambda: qkv_tile(6)],
                (0, 3): [lambda: c_proj(0, 0, RT)],
                (0, 7): [lambda: c_proj(1, 0, RT)],
            })
            attn_window(0, 1, 1, {
                (1, 9): [lambda: qkv_tile(7)],
                (1, 11): [lambda: c_proj(2, 0, RT)],
            })
            # b1 h0: finish b0 c_proj (fills the ACT-bound window)
            attn_window(1, 0, 0, {(0, 3): [lambda: c_proj(3, 0, RT)]})
            attn_window(1, 0, 1, {})
            # b1 h1: c_proj for b1 rows as they complete; fine-grained tail
            attn_window(1, 1, 0, {
                (0, 3): [lambda: c_proj(4, 0, RT)],
                (0, 7): [lambda: c_proj(5, 0, RT)],
            })
            attn_window(1, 1, 1, {
                (1, 11): [lambda: c_proj(6, 0, RT)],
                (1, 13): [lambda: c_proj(7, 0, 256)],
                (1, 15): [lambda: c_proj(7, 256, RT, tail=True)],
            })

            lp.__exit__(None, None, None)

    nc.compile()
    return nc


_NC = None


def _get_nc():
    global _NC
    if _NC is None:
        _NC = build_program()
    return _NC


def make_in_maps(x, W_attn, b_attn, W_proj, b_proj):
    x = np.asarray(x, np.float32)
    W_attn = np.asarray(W_attn, np.float32)
    b_attn = np.asarray(b_attn, np.float32)
    W_proj = np.asarray(W_proj, np.float32)
    b_proj = np.asarray(b_proj, np.float32)

    xT16 = np.ascontiguousarray(x.reshape(R, C).T).astype(np.float16)
    # tri[kk, j]: 1 if j >= kk (keep lower-triangular attention in S^T layout)
    tri = np.triu(np.ones((KA, KA), np.float16))

    Wq, Wk, Wv = (W_attn[:, i * C:(i + 1) * C] for i in range(3))
    bq, bk, bv_full = (b_attn[i * C:(i + 1) * C] for i in range(3))

    in_maps = []
    for core in range(NCORES):
        cols = slice(core * LC, (core + 1) * LC)
        wqk_l = np.concatenate([Wq[:, cols], Wk[:, cols]], axis=1)
        bqk_l = np.concatenate([bq[cols], bk[cols]])
        wv_l = Wv[:, cols].reshape(C, HL, HS)
        in_maps.append({
            "xT": xT16,
            "wqk": np.ascontiguousarray(wqk_l).astype(np.float16),
            "wv": np.ascontiguousarray(wv_l).astype(np.float16),
            "wp": np.ascontiguousarray(W_proj[cols, :]),
            "bqk": np.ascontiguousarray(bqk_l),
            "bv": np.ascontiguousarray(bv_full[cols]).astype(np.float16).reshape(1, LC),
            "trimask": tri,
        })
    return in_maps


def kernel(x, W_attn, b_attn, W_proj, b_proj):
    nc = _get_nc()
    in_maps = make_in_maps(x, W_attn, b_attn, W_proj, b_proj)
    res = run_bass_kernel_spmd(nc, in_maps, list(range(NCORES)))
    acc = res.results[0]["outT"].astype(np.float32)
    for corer in res.results[1:]:
        acc += corer["outT"].astype(np.float32)
    out = acc.T.reshape(B, T, C) + np.asarray(b_proj, np.float32)
    return out
